# revision 7
# baseline (speedup 1.0000x reference)
"""DPAP2PNet point-proposal head on 8 Trainium2 NeuronCores.

Strategy (hardcoded for the fixed problem size B=2, C=256, h=w=1024):
  - 4096 anchor points per batch image on a fixed 64x64 grid (stride 16, offset 8).
  - Shard by (batch, point-range): core 4*b + k handles points [k*1024, (k+1)*1024)
    of batch b. Each core only needs its batch's feature maps.
  - Feature maps are staged channel-last as one concatenated [87040, 256] DRAM
    table per batch (levels stacked row-major), so each bilinear corner is one
    contiguous row gathered with indirect DMA.
  - roi0 (anchor sampling from feat0) uses host-precomputed constant indices and
    weights. The deformed-point sampling computes indices/weights on device.
  - MLPs run on the PE with fp32(r) precision; bilinear combines on the DVE.
Output: [2, 4096, 9] float32 (pred coords ++ logits), assembled on host.
"""

from contextlib import ExitStack

import numpy as np

import concourse.bass as bass
import concourse.mybir as mybir
import concourse.tile as tile
from concourse import bacc
from concourse.bass_utils import run_bass_kernel_spmd
from concourse.masks import make_identity

F32 = mybir.dt.float32
F32R = mybir.dt.float32r
I32 = mybir.dt.int32
AOP = mybir.AluOpType
ACT = mybir.ActivationFunctionType

B, C, IMG = 2, 256, 1024
N_PTS = 4096            # per batch
N_CORES = 8
PTS_CORE = 1024         # per core (one batch)
G_TILES = 8             # 1024 / 128 point tiles per core
# (W, row base) per FPN level inside the per-batch gather table; stride*W = 1024.
LVL = [(256, 0), (128, 65536), (64, 81920), (32, 86016)]
TOTAL_ROWS = 87040

LAST_RESULTS = None     # BassKernelResults of the most recent run (for test.py)
TRACE = False           # set True to request an NTFF profile


# ----------------------------------------------------------------- host consts

def anchor_points():
    """[4096, 2] float32, point n -> (x, y) = ((n%64)*16+8, (n//64)*16+8)."""
    n = np.arange(N_PTS)
    return np.stack([(n % 64) * 16 + 8, (n // 64) * 16 + 8], axis=1).astype(np.float32)


def bilinear_tables(px, py, W):
    """Corner rows+weights for points (px,py) sampled on a WxW level.

    Returns idx [n,4] int32 (row = clip(y)*W+clip(x)) and w [n,4] float32 with
    the zero-padding validity mask folded in. Corner order: (x0,y0),(x1,y0),
    (x0,y1),(x1,y1). Matches reference.grid_sample_points exactly.
    """
    c = np.float32((W - 1) / 1024.0)
    x = (px * c).astype(np.float32)
    y = (py * c).astype(np.float32)
    x0 = np.floor(x); y0 = np.floor(y)
    fx = x - x0; fy = y - y0
    idx, wts = [], []
    for (cx, cy, wx, wy) in [
        (x0, y0, 1 - fx, 1 - fy), (x0 + 1, y0, fx, 1 - fy),
        (x0, y0 + 1, 1 - fx, fy), (x0 + 1, y0 + 1, fx, fy),
    ]:
        valid = (cx >= 0) & (cx <= W - 1) & (cy >= 0) & (cy <= W - 1)
        ix = np.clip(cx, 0, W - 1); iy = np.clip(cy, 0, W - 1)
        idx.append((iy * W + ix).astype(np.int32))
        wts.append((wx * wy * valid).astype(np.float32))
    return np.stack(idx, 1), np.stack(wts, 1)


def host_tables_for_slice(k):
    """Per-core constant tensors for point range [k*1024, (k+1)*1024)."""
    pts = anchor_points()[k * PTS_CORE:(k + 1) * PTS_CORE]  # [1024, 2]
    idx, wts = bilinear_tables(pts[:, 0], pts[:, 1], 256)   # level 0
    # layout [128, 32]: row p, col g*4+c  <->  point g*128+p, corner c
    idx0 = idx.reshape(G_TILES, 128, 4).transpose(1, 0, 2).reshape(128, 32)
    w0 = wts.reshape(G_TILES, 128, 4).transpose(1, 0, 2).reshape(128, 32)
    # canchorT [3, 1024]: rows (ones, x, y); col g*128+p <-> point g*128+p
    canchorT = np.concatenate([np.ones((1024, 1), np.float32), pts], axis=1).T
    return (np.ascontiguousarray(idx0), np.ascontiguousarray(w0),
            np.ascontiguousarray(canchorT))


def pack_weights(i):
    """Host-side repacks of the MLP weights (pure marshalling)."""
    w2bdef = np.concatenate([i["b_def2"][None, :], np.eye(2, dtype=np.float32)], 0)
    w1cat = np.concatenate([i["w_reg1"], i["w_cls1"]], axis=1)       # [1024, 512]
    b1cat = np.concatenate([i["b_reg1"], i["b_cls1"]])[:, None]      # [512, 1]
    w2stk = np.zeros((513, 10), np.float32)
    w2stk[0:256, 0:2] = i["w_reg2"]
    w2stk[256:512, 2:9] = i["w_cls2"]
    w2stk[512, 0:2] = i["b_reg2"]
    w2stk[512, 2:9] = i["b_cls2"]
    return {
        "wdef1": np.ascontiguousarray(i["w_def1"].astype(np.float32)),
        "bdef1": np.ascontiguousarray(i["b_def1"][:, None].astype(np.float32)),
        "wdef2": np.ascontiguousarray(i["w_def2"].astype(np.float32)),
        "w2bdef": np.ascontiguousarray(w2bdef.astype(np.float32)),
        "w1cat": np.ascontiguousarray(w1cat.astype(np.float32)),
        "b1cat": np.ascontiguousarray(b1cat.astype(np.float32)),
        "w2stk": np.ascontiguousarray(w2stk.astype(np.float32)),
    }


def featcat_for_batch(feats, b):
    """Concatenate channel-last level tables: [87040, 256] float32."""
    parts = [np.ascontiguousarray(np.transpose(f[b], (1, 2, 0)).reshape(-1, C))
             for f in feats]
    return np.concatenate(parts, axis=0)


# -------------------------------------------------------------- device program

def _emit(tc, ctx, t):
    nc = tc.nc

    const = ctx.enter_context(tc.tile_pool(name="const", bufs=1))
    p_roi0c = ctx.enter_context(tc.tile_pool(name="roi0c", bufs=2))
    p_roi0T = ctx.enter_context(tc.tile_pool(name="roi0T", bufs=1))
    p_hdefT = ctx.enter_context(tc.tile_pool(name="hdefT", bufs=1))
    p_dp = ctx.enter_context(tc.tile_pool(name="dp", bufs=1))
    p_coord = ctx.enter_context(tc.tile_pool(name="coord", bufs=2))
    p_iw = ctx.enter_context(tc.tile_pool(name="iw", bufs=1))
    p_lg = ctx.enter_context(tc.tile_pool(name="lg", bufs=2))
    p_flat = ctx.enter_context(tc.tile_pool(name="flat", bufs=2))
    p_fT = ctx.enter_context(tc.tile_pool(name="fT", bufs=1))
    p_hT = ctx.enter_context(tc.tile_pool(name="hT", bufs=1))
    p_out = ctx.enter_context(tc.tile_pool(name="out", bufs=2))
    ps_t = ctx.enter_context(tc.tile_pool(name="ps_t", bufs=2, space="PSUM"))
    ps_mm = ctx.enter_context(tc.tile_pool(name="ps_mm", bufs=2, space="PSUM"))
    ps_sm = ctx.enter_context(tc.tile_pool(name="ps_sm", bufs=2, space="PSUM"))

    def load(name, shape, dtype=F32):
        tl = const.tile(list(shape), dtype, tag=name)
        nc.sync.dma_start(tl[:], t[name])
        return tl

    ident = const.tile([128, 128], F32, tag="ident")
    make_identity(nc, ident[:])

    idx0 = load("idx0", [128, 32], I32)
    w0 = load("w0", [128, 32])
    canchorT = load("canchorT", [3, 1024], F32R)
    w2bdef = load("w2bdef", [3, 2], F32R)
    wdef1 = [const.tile([128, 256], F32R, name=f"wdef1_{k}", tag=f"wdef1_{k}") for k in range(2)]
    bdef1 = [const.tile([128, 1], F32, name=f"bdef1_{k}", tag=f"bdef1_{k}") for k in range(2)]
    wdef2 = [const.tile([128, 2], F32R, name=f"wdef2_{k}", tag=f"wdef2_{k}") for k in range(2)]
    for k in range(2):
        nc.sync.dma_start(wdef1[k][:], t["wdef1"][k * 128:(k + 1) * 128, :])
        nc.sync.dma_start(bdef1[k][:], t["bdef1"][k * 128:(k + 1) * 128, :])
        nc.sync.dma_start(wdef2[k][:], t["wdef2"][k * 128:(k + 1) * 128, :])
    w1cat = [const.tile([128, 512], F32R, name=f"w1cat_{k}", tag=f"w1cat_{k}") for k in range(8)]
    for k in range(8):
        nc.sync.dma_start(w1cat[k][:], t["w1cat"][k * 128:(k + 1) * 128, :])
    b1cat = [const.tile([128, 1], F32, name=f"b1cat_{k}", tag=f"b1cat_{k}") for k in range(4)]
    for k in range(4):
        nc.sync.dma_start(b1cat[k][:], t["b1cat"][k * 128:(k + 1) * 128, :])
    w2stk = [const.tile([128, 10], F32R, name=f"w2stk_{k}", tag=f"w2stk_{k}") for k in range(4)]
    for k in range(4):
        nc.sync.dma_start(w2stk[k][:], t["w2stk"][k * 128:(k + 1) * 128, :])
    w2r = const.tile([1, 10], F32R, tag="w2r")
    nc.sync.dma_start(w2r[:], t["w2stk"][512:513, :])

    featcat = t["featcat"]

    # --- phase A: roi0 gather + bilinear combine + transpose ------------------
    roi0T = [p_roi0T.tile([128, 1024], F32R, name=f"roi0T_{j}", tag=f"roi0T_{j}") for j in range(2)]
    for g in range(G_TILES):
        gsl = slice(g * 128, (g + 1) * 128)
        cor = p_roi0c.tile([128, 1024], F32, tag="cor")
        nc.gpsimd.indirect_dma_start(
            out=cor[:], out_offset=None, in_=featcat,
            in_offset=bass.IndirectOffsetOnAxis(ap=idx0[:, g * 4:(g + 1) * 4], axis=0),
        )
        acc = p_roi0c.tile([128, 256], F32, tag="racc")
        nc.vector.tensor_scalar_mul(acc[:], cor[:, 0:256], w0[:, g * 4:g * 4 + 1])
        for cidx in range(1, 4):
            nc.vector.scalar_tensor_tensor(
                out=acc[:], in0=cor[:, cidx * 256:(cidx + 1) * 256],
                scalar=w0[:, g * 4 + cidx:g * 4 + cidx + 1], in1=acc[:],
                op0=AOP.mult, op1=AOP.add)
        for j in range(2):
            ps = ps_t.tile([128, 128], F32, tag="tp")
            nc.tensor.transpose(ps[:], acc[:, j * 128:(j + 1) * 128], ident[:])
            nc.vector.tensor_copy(roi0T[j][:, gsl], ps[:])

    # --- phase B: deformation MLP hidden ------------------------------------
    hdefT = [p_hdefT.tile([128, 1024], F32R, name=f"hdefT_{m}", tag=f"hdefT_{m}") for m in range(2)]
    for m in range(2):
        for n2 in range(2):
            nsl = slice(n2 * 512, (n2 + 1) * 512)
            ps = ps_mm.tile([128, 512], F32, tag="mm")
            for k in range(2):
                nc.tensor.matmul(ps[:], wdef1[k][:, m * 128:(m + 1) * 128],
                                 roi0T[k][:, nsl], start=(k == 0), stop=(k == 1))
            nc.scalar.activation(hdefT[m][:, nsl], ps[:], ACT.Relu, bias=bdef1[m][:])

    # --- phase C: dp = anchors + delta  (dp_all[:, 2g:2g+2]) -----------------
    dp_all = p_dp.tile([128, 16], F32, tag="dp_all")
    for g in range(G_TILES):
        gsl = slice(g * 128, (g + 1) * 128)
        ps = ps_sm.tile([128, 10], F32, tag="sm")
        for k in range(2):
            nc.tensor.matmul(ps[:, 0:2], hdefT[k][:, gsl], wdef2[k][:],
                             start=(k == 0), stop=False)
        nc.tensor.matmul(ps[:, 0:2], canchorT[:, gsl], w2bdef[:],
                         start=False, stop=True)
        nc.vector.tensor_copy(dp_all[:, 2 * g:2 * g + 2], ps[:, 0:2])

    # --- phase D: per-level indices + weights --------------------------------
    IDX = p_iw.tile([128, 128], I32, tag="IDX")   # col = g*16 + l*4 + c
    WTS = p_iw.tile([128, 128], F32, tag="WTS")
    vx = dp_all[:, 0::2]   # [128, 8] x coords (col g)
    vy = dp_all[:, 1::2]

    def axis_pipeline(v, W, tagp):
        """returns (w0', w1', i0c, i1c) as [128, 8] tiles; weights mask-folded."""
        cl = float((W - 1) / 1024.0)
        x = p_coord.tile([128, 8], F32, tag=tagp + "x")
        nc.vector.tensor_scalar_mul(x[:], v, cl)
        xi = p_coord.tile([128, 8], I32, tag=tagp + "xi")
        nc.vector.tensor_copy(xi[:], x[:])
        xf = p_coord.tile([128, 8], F32, tag=tagp + "xf")
        nc.vector.tensor_copy(xf[:], xi[:])
        d = p_coord.tile([128, 8], F32, tag=tagp + "d")
        nc.vector.tensor_tensor(d[:], xf[:], x[:], op=AOP.is_gt)
        x0 = p_coord.tile([128, 8], F32, tag=tagp + "x0")
        nc.vector.tensor_tensor(x0[:], xf[:], d[:], op=AOP.subtract)
        fx = p_coord.tile([128, 8], F32, tag=tagp + "fx")
        nc.vector.tensor_tensor(fx[:], x[:], x0[:], op=AOP.subtract)
        b0 = p_coord.tile([128, 8], F32, tag=tagp + "b0")
        nc.vector.tensor_scalar(out=b0[:], in0=x0[:], scalar1=float(W - 1),
                                scalar2=None, op0=AOP.is_le)
        v0 = p_coord.tile([128, 8], F32, tag=tagp + "v0")
        nc.vector.scalar_tensor_tensor(out=v0[:], in0=x0[:], scalar=0.0, in1=b0[:],
                                       op0=AOP.is_ge, op1=AOP.mult)
        b1 = p_coord.tile([128, 8], F32, tag=tagp + "b1")
        nc.vector.tensor_scalar(out=b1[:], in0=x0[:], scalar1=float(W - 2),
                                scalar2=None, op0=AOP.is_le)
        v1 = p_coord.tile([128, 8], F32, tag=tagp + "v1")
        nc.vector.scalar_tensor_tensor(out=v1[:], in0=x0[:], scalar=-1.0, in1=b1[:],
                                       op0=AOP.is_ge, op1=AOP.mult)
        u = p_coord.tile([128, 8], F32, tag=tagp + "u")
        nc.vector.tensor_scalar(out=u[:], in0=fx[:], scalar1=-1.0, scalar2=1.0,
                                op0=AOP.mult, op1=AOP.add)
        w0_ = p_coord.tile([128, 8], F32, tag=tagp + "w0")
        nc.vector.tensor_tensor(w0_[:], u[:], v0[:], op=AOP.mult)
        w1_ = p_coord.tile([128, 8], F32, tag=tagp + "w1")
        nc.vector.tensor_tensor(w1_[:], fx[:], v1[:], op=AOP.mult)
        i0 = p_coord.tile([128, 8], F32, tag=tagp + "i0")
        nc.vector.tensor_scalar(out=i0[:], in0=x0[:], scalar1=0.0,
                                scalar2=float(W - 1), op0=AOP.max, op1=AOP.min)
        i1t = p_coord.tile([128, 8], F32, tag=tagp + "i1t")
        nc.vector.tensor_scalar(out=i1t[:], in0=x0[:], scalar1=-1.0,
                                scalar2=float(W - 2), op0=AOP.max, op1=AOP.min)
        i1 = p_coord.tile([128, 8], F32, tag=tagp + "i1")
        nc.vector.tensor_scalar(out=i1[:], in0=i1t[:], scalar1=1.0, scalar2=None,
                                op0=AOP.add)
        return w0_, w1_, i0, i1

    for l, (W, base) in enumerate(LVL):
        wx0, wx1, ix0, ix1 = axis_pipeline(vx, W, "ax")
        wy0, wy1, iy0, iy1 = axis_pipeline(vy, W, "ay")
        yW0 = p_coord.tile([128, 8], F32, tag="yW0")
        nc.vector.tensor_scalar(out=yW0[:], in0=iy0[:], scalar1=float(W),
                                scalar2=float(base), op0=AOP.mult, op1=AOP.add)
        yW1 = p_coord.tile([128, 8], F32, tag="yW1")
        nc.vector.tensor_scalar(out=yW1[:], in0=iy1[:], scalar1=float(W),
                                scalar2=float(base), op0=AOP.mult, op1=AOP.add)
        corners = [(yW0, ix0, wy0, wx0), (yW0, ix1, wy0, wx1),
                   (yW1, ix0, wy1, wx0), (yW1, ix1, wy1, wx1)]
        for cidx, (yW, ix, wy, wx) in enumerate(corners):
            col = l * 4 + cidx
            nc.vector.tensor_tensor(IDX[:, col::16], yW[:], ix[:], op=AOP.add)
            nc.vector.tensor_tensor(WTS[:, col::16], wy[:], wx[:], op=AOP.mult)

    # --- phase E/F/G: gather, combine, transpose -----------------------------
    fT = [p_fT.tile([128, 1024], F32R, name=f"fT_{j}", tag=f"fT_{j}") for j in range(8)]
    for g in range(G_TILES):
        gsl = slice(g * 128, (g + 1) * 128)
        lg = p_lg.tile([128, 4096], F32, tag="lg")
        nc.gpsimd.indirect_dma_start(
            out=lg[:], out_offset=None, in_=featcat,
            in_offset=bass.IndirectOffsetOnAxis(ap=IDX[:, g * 16:(g + 1) * 16], axis=0),
        )
        flat = p_flat.tile([128, 1024], F32, tag="flat")
        for l in range(4):
            sl = flat[:, l * 256:(l + 1) * 256]
            bcol = g * 16 + l * 4
            nc.vector.tensor_scalar_mul(
                sl, lg[:, (l * 4) * 256:(l * 4 + 1) * 256], WTS[:, bcol:bcol + 1])
            for cidx in range(1, 4):
                nc.vector.scalar_tensor_tensor(
                    out=sl, in0=lg[:, (l * 4 + cidx) * 256:(l * 4 + cidx + 1) * 256],
                    scalar=WTS[:, bcol + cidx:bcol + cidx + 1], in1=sl,
                    op0=AOP.mult, op1=AOP.add)
        for j in range(8):
            ps = ps_t.tile([128, 128], F32, tag="tp")
            nc.tensor.transpose(ps[:], flat[:, j * 128:(j + 1) * 128], ident[:])
            nc.vector.tensor_copy(fT[j][:, gsl], ps[:])

    # --- phase H: big MLP hidden ---------------------------------------------
    hT = [p_hT.tile([128, 1024], F32R, name=f"hT_{m}", tag=f"hT_{m}") for m in range(4)]
    for m in range(4):
        for n2 in range(2):
            nsl = slice(n2 * 512, (n2 + 1) * 512)
            ps = ps_mm.tile([128, 512], F32, tag="mm")
            for k in range(8):
                nc.tensor.matmul(ps[:], w1cat[k][:, m * 128:(m + 1) * 128],
                                 fT[k][:, nsl], start=(k == 0), stop=(k == 7))
            nc.scalar.activation(hT[m][:, nsl], ps[:], ACT.Relu, bias=b1cat[m][:])

    # --- phase I: heads + bias + dp add + store ------------------------------
    for g in range(G_TILES):
        gsl = slice(g * 128, (g + 1) * 128)
        ps = ps_sm.tile([128, 10], F32, tag="sm")
        for k in range(4):
            nc.tensor.matmul(ps[:], hT[k][:, gsl], w2stk[k][:],
                             start=(k == 0), stop=False)
        nc.tensor.matmul(ps[:], canchorT[0:1, gsl], w2r[:],
                         start=False, stop=True)
        osb = p_out.tile([128, 9], F32, tag="osb")
        nc.vector.tensor_copy(osb[:], ps[:, 0:9])
        nc.vector.tensor_tensor(osb[:, 0:2], osb[:, 0:2], dp_all[:, 2 * g:2 * g + 2],
                                op=AOP.add)
        nc.sync.dma_start(t["out"][gsl, :], osb[:])


def build_program():
    nc = bacc.Bacc("TRN2", target_bir_lowering=False, debug=False)
    t = {
        "featcat": nc.dram_tensor("featcat", [TOTAL_ROWS, C], F32, kind="ExternalInput").ap(),
        "idx0": nc.dram_tensor("idx0", [128, 32], I32, kind="ExternalInput").ap(),
        "w0": nc.dram_tensor("w0", [128, 32], F32, kind="ExternalInput").ap(),
        "canchorT": nc.dram_tensor("canchorT", [3, 1024], F32R, kind="ExternalInput").ap(),
        "wdef1": nc.dram_tensor("wdef1", [256, 256], F32R, kind="ExternalInput").ap(),
        "bdef1": nc.dram_tensor("bdef1", [256, 1], F32, kind="ExternalInput").ap(),
        "wdef2": nc.dram_tensor("wdef2", [256, 2], F32R, kind="ExternalInput").ap(),
        "w2bdef": nc.dram_tensor("w2bdef", [3, 2], F32R, kind="ExternalInput").ap(),
        "w1cat": nc.dram_tensor("w1cat", [1024, 512], F32R, kind="ExternalInput").ap(),
        "b1cat": nc.dram_tensor("b1cat", [512, 1], F32, kind="ExternalInput").ap(),
        "w2stk": nc.dram_tensor("w2stk", [513, 10], F32R, kind="ExternalInput").ap(),
        "out": nc.dram_tensor("out", [PTS_CORE, 9], F32, kind="ExternalOutput").ap(),
    }
    with tile.TileContext(nc) as tc, ExitStack() as ctx:
        _emit(tc, ctx, t)
    nc.compile()
    return nc


_PROG = None


def _get_program():
    global _PROG
    if _PROG is None:
        _PROG = build_program()
    return _PROG


# ------------------------------------------------------------------ entrypoint

def host_in_maps(feat0, feat1, feat2, feat3,
                 w_def1, b_def1, w_def2, b_def2,
                 w_reg1, b_reg1, w_reg2, b_reg2,
                 w_cls1, b_cls1, w_cls2, b_cls2,
                 h, w):
    assert int(h) == IMG and int(w) == IMG
    feats = [np.asarray(f, dtype=np.float32) for f in (feat0, feat1, feat2, feat3)]
    wk = pack_weights({
        "w_def1": np.asarray(w_def1), "b_def1": np.asarray(b_def1),
        "w_def2": np.asarray(w_def2), "b_def2": np.asarray(b_def2),
        "w_reg1": np.asarray(w_reg1), "b_reg1": np.asarray(b_reg1),
        "w_reg2": np.asarray(w_reg2), "b_reg2": np.asarray(b_reg2),
        "w_cls1": np.asarray(w_cls1), "b_cls1": np.asarray(b_cls1),
        "w_cls2": np.asarray(w_cls2), "b_cls2": np.asarray(b_cls2),
    })
    featcats = [featcat_for_batch(feats, b) for b in range(B)]
    slices = [host_tables_for_slice(k) for k in range(4)]

    in_maps = []
    for core in range(N_CORES):
        b, k = divmod(core, 4)
        idx0, w0, canchorT = slices[k]
        in_maps.append({
            "featcat": featcats[b], "idx0": idx0, "w0": w0, "canchorT": canchorT,
            **wk,
        })
    return in_maps


def kernel(**inputs):
    global LAST_RESULTS
    in_maps = host_in_maps(**inputs)
    nc = _get_program()
    res = run_bass_kernel_spmd(nc, in_maps, core_ids=list(range(N_CORES)),
                               trace=TRACE)
    LAST_RESULTS = res
    out = np.empty((B, N_PTS, 9), np.float32)
    for core in range(N_CORES):
        b, k = divmod(core, 4)
        out[b, k * PTS_CORE:(k + 1) * PTS_CORE, :] = res.results[core]["out"]
    return out


# revision 25
# speedup vs baseline: 7.9402x; 7.9402x over previous
"""DPAP2PNet point-proposal head on 8 Trainium2 NeuronCores.

Strategy (hardcoded for the fixed problem size B=2, C=256, h=w=1024):
  - 4096 anchor points per batch image on a fixed 64x64 grid (stride 16, offset 8).
  - Shard by (batch, point-range): core 4*b + k handles points [k*1024, (k+1)*1024)
    of batch b. Each core only needs its batch's feature maps.
  - Feature maps are staged channel-last bf16 as one concatenated [87040, 256]
    DRAM table per batch (levels stacked row-major); every bilinear corner is one
    contiguous 512B row gathered with indirect DMA.
  - roi0 (anchor sampling from feat0) uses host-precomputed constant indices and
    weights; deformed-point indices/weights are computed on device in fp32.
  - MLPs and transposes run on the PE in bf16; bilinear combines on the DVE.
Output: [2, 4096, 9] float32 (pred coords ++ logits), assembled on host.
"""

from contextlib import ExitStack

import numpy as np

import concourse.bass as bass
import concourse.mybir as mybir
import concourse.tile as tile
from concourse import bacc
from concourse.bass_utils import run_bass_kernel_spmd
from concourse.masks import make_identity

F32 = mybir.dt.float32
I32 = mybir.dt.int32
BF16 = mybir.dt.bfloat16
AOP = mybir.AluOpType
ACT = mybir.ActivationFunctionType

B, C, IMG = 2, 256, 1024
N_PTS = 4096            # per batch
N_CORES = 8
PTS_CORE = 1024         # per core (one batch)
G_TILES = 8             # 1024 / 128 point tiles per core
# (W, row base) per FPN level inside the per-batch gather table; stride*W = 1024.
LVL = [(256, 0), (128, 65536), (64, 81920), (32, 86016)]
TOTAL_ROWS = 87040

LAST_RESULTS = None     # BassKernelResults of the most recent run (for test.py)
TRACE = False

# fconst layout (f32 [128, 320])
FC_W0 = 0        # 32: roi0 bilinear weights
FC_ANCH = 32     # 16: anchors (+b_def2) in dp layout
FC_LVL = 48      # 320: CL2(64) WM1(64) WM2(64) WW(32) BASE(32)
FC_B2 = 304      # 10: head bias broadcast
FC_BD1 = 314     # 2: def-MLP bias columns
FC_B1 = 316     # 4: big-MLP bias columns
FC_W = 320
# wbf layout (bf16 [128, 4652])
WB_W1 = 0        # 8 k-tiles x 512
WB_WD1 = 4096    # 2 k-tiles x 256
WB_WD2 = 4608    # 2 k-tiles x 2
WB_W2 = 4612     # 4 k-tiles x 10
WB_W = 4652


# ----------------------------------------------------------------- host consts

def anchor_points():
    """[4096, 2] float32, point n -> (x, y) = ((n%64)*16+8, (n//64)*16+8)."""
    n = np.arange(N_PTS)
    return np.stack([(n % 64) * 16 + 8, (n // 64) * 16 + 8], axis=1).astype(np.float32)


def bilinear_tables(px, py, W):
    """Corner rows+weights for points (px,py) sampled on a WxW level.

    Returns idx [n,4] int32 (row = clip(y)*W+clip(x)) and w [n,4] float32 with
    the zero-padding validity mask folded in. Corner order: (x0,y0),(x1,y0),
    (x0,y1),(x1,y1). Matches reference.grid_sample_points exactly.
    """
    c = np.float32((W - 1) / 1024.0)
    x = (px * c).astype(np.float32)
    y = (py * c).astype(np.float32)
    x0 = np.floor(x); y0 = np.floor(y)
    fx = x - x0; fy = y - y0
    idx, wts = [], []
    for (cx, cy, wx, wy) in [
        (x0, y0, 1 - fx, 1 - fy), (x0 + 1, y0, fx, 1 - fy),
        (x0, y0 + 1, 1 - fx, fy), (x0 + 1, y0 + 1, fx, fy),
    ]:
        valid = (cx >= 0) & (cx <= W - 1) & (cy >= 0) & (cy <= W - 1)
        ix = np.clip(cx, 0, W - 1); iy = np.clip(cy, 0, W - 1)
        idx.append((iy * W + ix).astype(np.int32))
        wts.append((wx * wy * valid).astype(np.float32))
    return np.stack(idx, 1), np.stack(wts, 1)


def level_consts2():
    """[128, 256] f32: CL2(64) WM1(64) WM2(64) WW(32) BASE(32); col l*8+g."""
    def blk(vals):
        row = np.zeros((1, 32), np.float32)
        for l in range(4):
            row[0, l * 8:(l + 1) * 8] = vals[l]
        return row
    cl = blk([(W - 1) / 1024.0 for W, _ in LVL])
    wm1 = blk([W - 1 for W, _ in LVL])
    wm2 = blk([W - 2 for W, _ in LVL])
    ww = blk([W for W, _ in LVL])
    base = blk([b for _, b in LVL])
    row = np.concatenate([cl, cl, wm1, wm1, wm2, wm2, ww, base], axis=1)
    return np.repeat(row, 128, axis=0)


def host_tables_for_slice(k):
    """(idx0 [128,32] i32, w0 [128,32] f32, anchpg [128,16] f32) for core k."""
    pts = anchor_points()[k * PTS_CORE:(k + 1) * PTS_CORE]  # [1024, 2]
    idx, wts = bilinear_tables(pts[:, 0], pts[:, 1], 256)   # level 0
    idx0 = idx.reshape(G_TILES, 128, 4).transpose(1, 0, 2).reshape(128, 32)
    w0 = wts.reshape(G_TILES, 128, 4).transpose(1, 0, 2).reshape(128, 32)
    anchpg = pts.reshape(G_TILES, 128, 2).transpose(1, 0, 2).reshape(128, 16)
    return (np.ascontiguousarray(idx0), np.ascontiguousarray(w0),
            np.ascontiguousarray(anchpg))


def pack_weights(i):
    """Host-side repacks of the MLP weights (pure marshalling)."""
    import ml_dtypes
    bf = ml_dtypes.bfloat16

    def ktile(w, nk):  # [nk*128, m] -> [128, nk*m]
        m = w.shape[1]
        return w.reshape(nk, 128, m).transpose(1, 0, 2).reshape(128, nk * m)

    w1cat = np.concatenate([i["w_reg1"], i["w_cls1"]], axis=1)       # [1024, 512]
    w2stk = np.zeros((512, 10), np.float32)
    w2stk[0:256, 0:2] = i["w_reg2"]
    w2stk[256:512, 2:9] = i["w_cls2"]
    wbf = np.concatenate([
        ktile(w1cat.astype(np.float32), 8),
        ktile(i["w_def1"].astype(np.float32), 2),
        ktile(i["w_def2"].astype(np.float32), 2),
        ktile(w2stk, 4),
    ], axis=1).astype(bf)
    assert wbf.shape == (128, WB_W)

    b2bc = np.zeros((1, 10), np.float32)
    b2bc[0, 0:2] = i["b_reg2"]
    b2bc[0, 2:9] = i["b_cls2"]
    fpart = np.concatenate([
        np.repeat(b2bc, 128, axis=0),
        i["b_def1"].reshape(2, 128).T.astype(np.float32),
        np.concatenate([i["b_reg1"], i["b_cls1"]]).reshape(4, 128).T
        .astype(np.float32),
    ], axis=1)  # [128, 16]
    return (np.ascontiguousarray(wbf), np.ascontiguousarray(fpart),
            np.tile(i["b_def2"].astype(np.float32), 8)[None, :])


def featcat_for_batch(feats, b):
    """Concatenate channel-last level tables: [87040, 256] bfloat16."""
    import ml_dtypes
    bf = ml_dtypes.bfloat16
    parts = [np.ascontiguousarray(np.transpose(f[b], (1, 2, 0)).reshape(-1, C)
                                  .astype(bf)) for f in feats]
    return np.concatenate(parts, axis=0)


def host_in_maps(feat0, feat1, feat2, feat3,
                 w_def1, b_def1, w_def2, b_def2,
                 w_reg1, b_reg1, w_reg2, b_reg2,
                 w_cls1, b_cls1, w_cls2, b_cls2,
                 h, w):
    assert int(h) == IMG and int(w) == IMG
    feats = [np.asarray(f, dtype=np.float32) for f in (feat0, feat1, feat2, feat3)]
    wbf, fpart, bdef2row = pack_weights({
        "w_def1": np.asarray(w_def1), "b_def1": np.asarray(b_def1),
        "w_def2": np.asarray(w_def2), "b_def2": np.asarray(b_def2),
        "w_reg1": np.asarray(w_reg1), "b_reg1": np.asarray(b_reg1),
        "w_reg2": np.asarray(w_reg2), "b_reg2": np.asarray(b_reg2),
        "w_cls1": np.asarray(w_cls1), "b_cls1": np.asarray(b_cls1),
        "w_cls2": np.asarray(w_cls2), "b_cls2": np.asarray(b_cls2),
    })
    featcats = [featcat_for_batch(feats, b) for b in range(B)]
    lvlc2 = level_consts2()

    in_maps = []
    for core in range(N_CORES):
        b, k = divmod(core, 4)
        idx0, w0, anchpg = host_tables_for_slice(k)
        fconst = np.concatenate([
            w0, (anchpg + bdef2row).astype(np.float32), lvlc2, fpart,
        ], axis=1)
        assert fconst.shape == (128, FC_W)
        in_maps.append({
            "featcat": featcats[b], "idx0": idx0,
            "fconst": np.ascontiguousarray(fconst), "wbf": wbf,
        })
    return in_maps


# -------------------------------------------------------------- device program

def _emit(tc, ctx, t):
    nc = tc.nc

    const = ctx.enter_context(tc.tile_pool(name="const", bufs=1))
    p_roi0c = ctx.enter_context(tc.tile_pool(name="roi0c", bufs=4))
    p_roi0T = ctx.enter_context(tc.tile_pool(name="roi0T", bufs=1))
    p_hdefT = ctx.enter_context(tc.tile_pool(name="hdefT", bufs=1))
    p_dp = ctx.enter_context(tc.tile_pool(name="dp", bufs=1))
    p_coord = ctx.enter_context(tc.tile_pool(name="coord", bufs=2))
    p_iw = ctx.enter_context(tc.tile_pool(name="iw", bufs=1))
    p_lg = ctx.enter_context(tc.tile_pool(name="lg", bufs=4))
    p_flat = ctx.enter_context(tc.tile_pool(name="flat", bufs=2))
    p_fT = ctx.enter_context(tc.tile_pool(name="fT", bufs=1))
    p_hT = ctx.enter_context(tc.tile_pool(name="hT", bufs=1))
    p_out = ctx.enter_context(tc.tile_pool(name="out", bufs=1))
    ps_t = ctx.enter_context(tc.tile_pool(name="ps_t", bufs=2, space="PSUM"))
    ps_mm = ctx.enter_context(tc.tile_pool(name="ps_mm", bufs=2, space="PSUM"))
    ps_sm = ctx.enter_context(tc.tile_pool(name="ps_sm", bufs=2, space="PSUM"))

    ident = const.tile([128, 128], BF16, tag="ident")
    make_identity(nc, ident[:])
    identf = const.tile([16, 16], F32, tag="identf")
    make_identity(nc, identf[:])

    idx0 = const.tile([128, 32], I32, tag="idx0")
    nc.sync.dma_start(idx0[:], t["idx0"])
    fc = const.tile([128, FC_W], F32, tag="fc")
    nc.sync.dma_start(fc[:], t["fconst"])
    wb = const.tile([128, WB_W], BF16, tag="wb")
    nc.sync.dma_start(wb[:], t["wbf"])

    w0 = fc[:, FC_W0:FC_W0 + 32]
    anchpg = fc[:, FC_ANCH:FC_ANCH + 16]
    CL2 = fc[:, FC_LVL:FC_LVL + 64]
    WM1 = fc[:, FC_LVL + 64:FC_LVL + 128]
    WM2 = fc[:, FC_LVL + 128:FC_LVL + 192]
    WWb = fc[:, FC_LVL + 192:FC_LVL + 224]
    BASEb = fc[:, FC_LVL + 224:FC_LVL + 256]
    b2bc = fc[:, FC_B2:FC_B2 + 10]
    bdef1 = [fc[:, FC_BD1 + m:FC_BD1 + m + 1] for m in range(2)]
    b1cat = [fc[:, FC_B1 + m:FC_B1 + m + 1] for m in range(4)]

    def w1cat(k, m):
        return wb[:, WB_W1 + k * 512 + m * 128:WB_W1 + k * 512 + (m + 1) * 128]

    def wdef1(k, m):
        return wb[:, WB_WD1 + k * 256 + m * 128:WB_WD1 + k * 256 + (m + 1) * 128]

    def wdef2(k):
        return wb[:, WB_WD2 + k * 2:WB_WD2 + (k + 1) * 2]

    def w2stk(k):
        return wb[:, WB_W2 + k * 10:WB_W2 + (k + 1) * 10]

    featcat = t["featcat"]

    # --- phase A: roi0 gather + bilinear combine + transpose -----------------
    roi0Tall = p_roi0T.tile([128, 2048], BF16, tag="roi0Tall")
    r0v = roi0Tall[:].rearrange("p (j q) -> p j q", j=2)
    cor_tiles = {}
    for gp in range(4):
        cor2 = p_roi0c.tile([128, 2048], BF16, tag="cor", name=f"cor_{gp}")
        nc.gpsimd.indirect_dma_start(
            out=cor2[:], out_offset=None, in_=featcat,
            in_offset=bass.IndirectOffsetOnAxis(
                ap=idx0[:, gp * 8:(gp + 1) * 8], axis=0),
        )
        cor_tiles[gp] = cor2
    for g in range(G_TILES):
        gsl = slice(g * 128, (g + 1) * 128)
        cor = cor_tiles[g // 2][:, (g % 2) * 1024:(g % 2) * 1024 + 1024]
        acc = p_roi0c.tile([128, 256], BF16, tag="racc")
        rtmp = p_roi0c.tile([128, 256], BF16, tag="rtmp")
        nc.vector.tensor_scalar_mul(acc[:], cor[:, 0:256], w0[:, g * 4:g * 4 + 1])
        for cidx in range(1, 4):
            nc.vector.tensor_scalar_mul(
                rtmp[:], cor[:, cidx * 256:(cidx + 1) * 256],
                w0[:, g * 4 + cidx:g * 4 + cidx + 1])
            nc.vector.tensor_tensor(acc[:], acc[:], rtmp[:], op=AOP.add)
        ps2 = ps_t.tile([128, 256], BF16, tag="tp")
        for j in range(2):
            nc.tensor.transpose(ps2[:, j * 128:(j + 1) * 128],
                                acc[:, j * 128:(j + 1) * 128], ident[:])
        nc.vector.tensor_copy(
            r0v[:, :, gsl], ps2[:].rearrange("p (j q) -> p j q", j=2))

    # --- phase B: deformation MLP hidden -------------------------------------
    hdefT = [p_hdefT.tile([128, 1024], BF16, name=f"hdefT_{m}", tag=f"hdefT_{m}")
             for m in range(2)]
    for n2 in range(2):
        for m in range(2):
            nsl = slice(n2 * 512, (n2 + 1) * 512)
            ps = ps_mm.tile([128, 512], F32, tag="mm")
            for k in range(2):
                nc.tensor.matmul(ps[:], wdef1(k, m),
                                 roi0Tall[:, k * 1024 + nsl.start:k * 1024 + nsl.stop],
                                 start=(k == 0), stop=(k == 1))
            nc.scalar.activation(hdefT[m][:, nsl], ps[:], ACT.Relu, bias=bdef1[m])

    # --- phase C: dp = anchors(+b_def2) + delta; split into g-halves --------
    dp_h = [p_dp.tile([128, 8], F32, name=f"dp_{h}", tag=f"dp_{h}")
            for h in range(2)]

    def emit_dp(g):
        gsl = slice(g * 128, (g + 1) * 128)
        ps = ps_sm.tile([128, 10], F32, tag="sm")
        for k in range(2):
            nc.tensor.matmul(ps[:, 0:2], hdefT[k][:, gsl], wdef2(k),
                             start=(k == 0), stop=(k == 1))
        q = g % 4
        nc.vector.tensor_tensor(dp_h[g // 4][:, 2 * q:2 * q + 2], ps[:, 0:2],
                                anchpg[:, 2 * g:2 * g + 2], op=AOP.add)



    # --- phase D: per-level indices + weights, per g-half --------------------
    IDXh = [p_iw.tile([128, 64], I32, name=f"IDX_{h}", tag=f"IDX_{h}")
            for h in range(2)]
    WTSh = [p_iw.tile([128, 64], F32, name=f"WTS_{h}", tag=f"WTS_{h}")
            for h in range(2)]

    def al4(v):   # [128, 32] (a l g) flat -> [128, 2, 4, 4]
        return v.rearrange("p (a l g) -> p a l g", a=2, l=4)

    def lg4(v):   # [128, 16] (l g) -> [128, 4, 4]
        return v.rearrange("p (l g) -> p l g", l=4)

    def glv(v):   # [128, 16] (l g) -> [128, g, l] view
        return v.rearrange("p (l g) -> p g l", l=4)

    def coords_half(h):
        hs = slice(h * 4, h * 4 + 4)
        cl_h = al4(CL2).copy()[:, :, :, hs]
        wm1_h = al4(WM1)[:, :, :, hs]
        wm2_h = al4(WM2)[:, :, :, hs]
        ww_h = lg4(WWb)[:, :, hs]
        base_h = lg4(BASEb)[:, :, hs]

        xy = p_coord.tile([128, 32], F32, tag="xy", name="xy")
        nc.vector.tensor_copy(
            xy[:, 0:16].rearrange("p (l g) -> p l g", l=4),
            dp_h[h][:, 0::2].unsqueeze(1).broadcast_to((128, 4, 4)))
        nc.vector.tensor_copy(
            xy[:, 16:32].rearrange("p (l g) -> p l g", l=4),
            dp_h[h][:, 1::2].unsqueeze(1).broadcast_to((128, 4, 4)))
        nc.vector.tensor_tensor(al4(xy[:]), al4(xy[:]), cl_h, op=AOP.mult)

        def ctile(tag, dtype=F32):
            return p_coord.tile([128, 32], dtype, tag=tag, name=tag)

        xi = ctile("xi", I32)
        nc.vector.tensor_copy(xi[:], xy[:])
        xf = ctile("xf")
        nc.vector.tensor_copy(xf[:], xi[:])
        d = ctile("d")
        nc.vector.tensor_tensor(d[:], xf[:], xy[:], op=AOP.is_gt)
        x0 = ctile("x0")
        nc.vector.tensor_tensor(x0[:], xf[:], d[:], op=AOP.subtract)
        fx = ctile("fx")
        nc.vector.tensor_tensor(fx[:], xy[:], x0[:], op=AOP.subtract)
        b0 = ctile("b0")
        nc.vector.tensor_tensor(al4(b0[:]), al4(x0[:]), wm1_h, op=AOP.is_le)
        a0 = ctile("a0")
        nc.vector.tensor_scalar(out=a0[:], in0=x0[:], scalar1=0.0, scalar2=None,
                                op0=AOP.is_ge)
        v0 = ctile("v0")
        nc.vector.tensor_tensor(v0[:], a0[:], b0[:], op=AOP.mult)
        b1 = ctile("b1")
        nc.vector.tensor_tensor(al4(b1[:]), al4(x0[:]), wm2_h, op=AOP.is_le)
        a1 = ctile("a1")
        nc.vector.tensor_scalar(out=a1[:], in0=x0[:], scalar1=-1.0, scalar2=None,
                                op0=AOP.is_ge)
        v1 = ctile("v1")
        nc.vector.tensor_tensor(v1[:], a1[:], b1[:], op=AOP.mult)
        u = ctile("u")
        nc.vector.tensor_scalar(out=u[:], in0=fx[:], scalar1=-1.0, scalar2=1.0,
                                op0=AOP.mult, op1=AOP.add)
        wq0 = ctile("wq0")
        nc.vector.tensor_tensor(wq0[:], u[:], v0[:], op=AOP.mult)
        wq1 = ctile("wq1")
        nc.vector.tensor_tensor(wq1[:], fx[:], v1[:], op=AOP.mult)
        i0a = ctile("i0a")
        nc.vector.tensor_scalar(out=i0a[:], in0=x0[:], scalar1=0.0, scalar2=None,
                                op0=AOP.max)
        i0 = ctile("i0")
        nc.vector.tensor_tensor(al4(i0[:]), al4(i0a[:]), wm1_h, op=AOP.min)
        i1a = ctile("i1a")
        nc.vector.tensor_scalar(out=i1a[:], in0=x0[:], scalar1=-1.0, scalar2=None,
                                op0=AOP.max)
        i1b = ctile("i1b")
        nc.vector.tensor_tensor(al4(i1b[:]), al4(i1a[:]), wm2_h, op=AOP.min)
        i1 = ctile("i1")
        nc.vector.tensor_scalar(out=i1[:], in0=i1b[:], scalar1=1.0, scalar2=None,
                                op0=AOP.add)

        yW0 = p_coord.tile([128, 16], F32, tag="yW0", name="yW0")
        nc.vector.tensor_tensor(lg4(yW0[:]), lg4(i0[:, 16:32]), ww_h, op=AOP.mult)
        nc.vector.tensor_tensor(lg4(yW0[:]), lg4(yW0[:]), base_h, op=AOP.add)
        yW1 = p_coord.tile([128, 16], F32, tag="yW1", name="yW1")
        nc.vector.tensor_tensor(lg4(yW1[:]), lg4(i1[:, 16:32]), ww_h, op=AOP.mult)
        nc.vector.tensor_tensor(lg4(yW1[:]), lg4(yW1[:]), base_h, op=AOP.add)

        corners = [(yW0[:], i0[:, 0:16], wq0[:, 16:32], wq0[:, 0:16]),
                   (yW0[:], i1[:, 0:16], wq0[:, 16:32], wq1[:, 0:16]),
                   (yW1[:], i0[:, 0:16], wq1[:, 16:32], wq0[:, 0:16]),
                   (yW1[:], i1[:, 0:16], wq1[:, 16:32], wq1[:, 0:16])]
        for cidx, (yW, ix, wy, wx) in enumerate(corners):
            dsti = IDXh[h][:, cidx::4].rearrange("p (g l) -> p g l", l=4)
            nc.vector.tensor_tensor(dsti, glv(yW), glv(ix), op=AOP.add)
            dstw = WTSh[h][:, cidx::4].rearrange("p (g l) -> p g l", l=4)
            nc.vector.tensor_tensor(dstw, glv(wy), glv(wx), op=AOP.mult)

    # --- phase E/F/G: gather, combine, transpose -----------------------------
    fTall = p_fT.tile([128, 8192], BF16, tag="fTall")
    fTv = fTall[:].rearrange("p (j q) -> p j q", j=8)
    lg_tiles = {}

    def emit_gather(gp):
        lg2 = p_lg.tile([128, 8192], BF16, tag="lg", name=f"lg_{gp}")
        nc.gpsimd.indirect_dma_start(
            out=lg2[:], out_offset=None, in_=featcat,
            in_offset=bass.IndirectOffsetOnAxis(
                ap=IDXh[gp // 2][:, (gp % 2) * 32:(gp % 2) * 32 + 32], axis=0),
        )
        lg_tiles[gp] = lg2

    for h in range(2):
        for g in range(4 * h, 4 * h + 4):
            emit_dp(g)
        coords_half(h)
        emit_gather(2 * h)
        emit_gather(2 * h + 1)
    for g in range(G_TILES):
        gsl = slice(g * 128, (g + 1) * 128)
        lg = lg_tiles[g // 2]
        flat = p_flat.tile([128, 1024], BF16, tag="flat")
        ctmp = p_flat.tile([128, 256], BF16, tag="ctmp")
        lgo = (g % 2) * 4096
        for l in range(4):
            sl = flat[:, l * 256:(l + 1) * 256]
            WTS = WTSh[g // 4]
            bcol = (g % 4) * 16 + l * 4
            nc.vector.tensor_scalar_mul(
                sl, lg[:, lgo + (l * 4) * 256:lgo + (l * 4 + 1) * 256],
                WTS[:, bcol:bcol + 1])
            for cidx in range(1, 4):
                nc.vector.tensor_scalar_mul(
                    ctmp[:],
                    lg[:, lgo + (l * 4 + cidx) * 256:lgo + (l * 4 + cidx + 1) * 256],
                    WTS[:, bcol + cidx:bcol + cidx + 1])
                nc.vector.tensor_tensor(sl, sl, ctmp[:], op=AOP.add)
        ps8 = ps_t.tile([128, 1024], BF16, tag="tp")
        for j in range(8):
            nc.tensor.transpose(ps8[:, j * 128:(j + 1) * 128],
                                flat[:, j * 128:(j + 1) * 128], ident[:])
        nc.scalar.activation(fTv[:, :, gsl],
                             ps8[:].rearrange("p (j q) -> p j q", j=8), ACT.Copy)

    # --- phase H: big MLP hidden ---------------------------------------------
    hT = [p_hT.tile([128, 1024], BF16, name=f"hT_{m}", tag=f"hT_{m}")
          for m in range(4)]
    for n2 in range(2):
        for m in range(4):
            nsl = slice(n2 * 512, (n2 + 1) * 512)
            ps = ps_mm.tile([128, 512], F32, tag="mm")
            for k in range(8):
                nc.tensor.matmul(ps[:], w1cat(k, m),
                                 fTall[:, k * 1024 + nsl.start:k * 1024 + nsl.stop],
                                 start=(k == 0), stop=(k == 7))
            nc.scalar.activation(hT[m][:, nsl], ps[:], ACT.Relu, bias=b1cat[m])

    # --- phase I: heads (transposed) + bias + dp add + one store -------------
    o2T = p_out.tile([10, 1024], F32, tag="o2T")
    for n2 in range(2):
        nsl = slice(n2 * 512, (n2 + 1) * 512)
        ps = ps_mm.tile([10, 512], F32, tag="mmo", bufs=2)
        for k in range(4):
            nc.tensor.matmul(ps[:], w2stk(k), hT[k][:, nsl],
                             start=(k == 0), stop=(k == 3))
        nc.vector.tensor_copy(o2T[:, nsl], ps[:])
    osb_all = p_out.tile([128, 72], F32, tag="osb_all")
    for g in range(G_TILES):
        gsl = slice(g * 128, (g + 1) * 128)
        pst = ps_sm.tile([128, 10], F32, tag="sm")
        nc.tensor.transpose(pst[:], o2T[:, gsl], identf[0:10, 0:10])
        osb = osb_all[:, g * 9:(g + 1) * 9]
        nc.vector.tensor_tensor(osb, pst[:, 0:9], b2bc[:, 0:9], op=AOP.add)
        nc.vector.tensor_tensor(osb[:, 0:2], osb[:, 0:2],
                                dp_h[g // 4][:, 2 * (g % 4):2 * (g % 4) + 2],
                                op=AOP.add)
    nc.sync.dma_start(
        t["out"].rearrange("(g p) j -> p g j", g=8),
        osb_all[:].rearrange("p (g j) -> p g j", j=9))


def build_program():
    nc = bacc.Bacc("TRN2", target_bir_lowering=False, debug=False)
    t = {
        "featcat": nc.dram_tensor("featcat", [TOTAL_ROWS, C], BF16,
                                  kind="ExternalInput").ap(),
        "idx0": nc.dram_tensor("idx0", [128, 32], I32, kind="ExternalInput").ap(),
        "fconst": nc.dram_tensor("fconst", [128, FC_W], F32,
                                 kind="ExternalInput").ap(),
        "wbf": nc.dram_tensor("wbf", [128, WB_W], BF16, kind="ExternalInput").ap(),
        "out": nc.dram_tensor("out", [PTS_CORE, 9], F32, kind="ExternalOutput").ap(),
    }
    with tile.TileContext(nc) as tc, ExitStack() as ctx:
        _emit(tc, ctx, t)
    nc.compile()
    return nc


_PROG = None


def _get_program():
    global _PROG
    if _PROG is None:
        _PROG = build_program()
    return _PROG


# ------------------------------------------------------------------ entrypoint

def kernel(**inputs):
    global LAST_RESULTS
    in_maps = host_in_maps(**inputs)
    nc = _get_program()
    res = run_bass_kernel_spmd(nc, in_maps, core_ids=list(range(N_CORES)),
                               trace=TRACE)
    LAST_RESULTS = res
    out = np.empty((B, N_PTS, 9), np.float32)
    for core in range(N_CORES):
        b, k = divmod(core, 4)
        out[b, k * PTS_CORE:(k + 1) * PTS_CORE, :] = res.results[core]["out"]
    return out
